# revision 3
# baseline (speedup 1.0000x reference)
"""Trainium2 Bass kernel for ComputeVecSimilarityLoss.

Reference semantics (B batches, N points, D=2):
    sm      = where(cos < th, 0, cos)                      [B,N,N]
    v[i,j]  = (gt[i] - gt[j]) * sm[i,j]  -> [B, M=N*N, D]
    dot     = v @ v^T per batch                            [B,M,M]
    idx_num = count(dot != 0)
    vabs    = sqrt(sum(v*v + 1e-9, axis=D))
    result  = sum(|dot| / (vabs_m*vabs_n)) / idx_num

Restructuring used here (mathematically exact, fp-equal to ~1e-6):
  * u = v / vabs  (host, O(B*M)): |dot|/(vabs_m*vabs_n) == |u_m . u_n|.
  * zero vectors (v == 0) contribute exactly 0.0 to every sum term and
    have dot == 0 for every pair, so they are compacted away on the host.
    idx_num = sum_b (nnz_b)^2  (the residual pairs have |dot| >> 0, no
    exact-cancellation; verified against the reference count).
  * batch b -> NeuronCore b (pure data parallel, B == 8 cores).
    Each core computes S_b = sum |u_m . u_n| over its compacted M_b x M_b
    block plus its nonzero count; host does the final scalar division.

Device kernel per core: PE emits K=2 matmuls (u^T u outer products) into
PSUM 256-column chunks; ScalarE (Abs activation with accum_out) and
VectorE (tensor_reduce with apply_absolute_value) split the |.|+sum work
over PSUM supertiles; per-partition partials are reduced on-chip to a
[128, 2] output (abs-dot partial sums, nz partial sums).
"""

import os

import numpy as np

EPS = np.float32(1e-9)
CHUNK = 256          # matmul free-dim per PSUM chunk (half bank of fp32)
GROUP_CHUNKS = 4     # chunks per PSUM supertile (2 banks = 1024 fp32)
N_CORES = 8

# Stash of the most recent BassKernelResults (for test harness profiling).
LAST_RESULTS = None

_PROGRAM_CACHE = {}


def _plan_groups(n_tiles, n_chunks):
    """Plan (work groups, consumer assignment, partials columns).

    Work item = (m_tile t, col chunk c).  Groups of up to GROUP_CHUNKS
    items share one PSUM supertile and one consumer instruction.
    Returns list of (items, engine, col0) and the partials width.
    """
    work = [(t, c) for t in range(n_tiles) for c in range(n_chunks)]
    groups = [work[i : i + GROUP_CHUNKS] for i in range(0, len(work), GROUP_CHUNKS)]

    plan = []
    col = 0
    act_ns = 0.0
    dve_ns = 0.0
    for items in groups:
        fd = CHUNK * len(items)
        cost_act = (172.0 + fd) / 1.2
        cost_dve = (120.0 + fd) / 0.96
        if act_ns + cost_act <= dve_ns + cost_dve:
            plan.append((items, "act", col))
            act_ns += cost_act
            col += 1
        else:
            plan.append((items, "dve", col))
            dve_ns += cost_dve
            col += len(items)
    return plan, col


def _build_program(P, COLS):
    """Build (and cache) the Bass program for padded size P x COLS."""
    key = (P, COLS)
    if key in _PROGRAM_CACHE:
        return _PROGRAM_CACHE[key]

    import concourse.bass as bass
    import concourse.mybir as mybir
    import concourse.tile as tile
    from concourse import bacc
    from concourse.bass import ts

    f32 = mybir.dt.float32
    n_tiles = P // 128
    n_chunks = COLS // CHUNK
    plan, npart = _plan_groups(n_tiles, n_chunks)

    nc = bacc.Bacc(
        "TRN2",
        target_bir_lowering=False,
        debug=False,
        enable_asserts=False,
        num_devices=N_CORES,
    )
    u_dram = nc.dram_tensor("u", [2, COLS], f32, kind="ExternalInput")
    nz_dram = nc.dram_tensor("nz", [128, n_tiles], f32, kind="ExternalInput")
    out_dram = nc.dram_tensor("out", [128, 2], f32, kind="ExternalOutput")

    with tile.TileContext(nc) as tc:
        with (
            tc.tile_pool(name="const", bufs=1) as const_pool,
            tc.tile_pool(name="psum", bufs=4, space="PSUM") as psum_pool,
        ):
            u = const_pool.tile([2, COLS], f32)
            nc.sync.dma_start(u[:], u_dram.ap())
            nzt = const_pool.tile([128, n_tiles], f32)
            nc.sync.dma_start(nzt[:], nz_dram.ap())
            partials = const_pool.tile([128, npart], f32)

            for items, engine, col0 in plan:
                w = CHUNK * len(items)
                ps = psum_pool.tile([128, GROUP_CHUNKS * CHUNK], f32, tag="ps")
                for j, (t, c) in enumerate(items):
                    nc.tensor.matmul(
                        ps[:, ts(j, CHUNK)],
                        u[:, ts(t, 128)],
                        u[:, ts(c, CHUNK)],
                    )
                if engine == "act":
                    nc.scalar.activation(
                        ps[:, 0:w],
                        ps[:, 0:w],
                        mybir.ActivationFunctionType.Abs,
                        accum_out=partials[:, col0 : col0 + 1],
                    )
                else:
                    nc.vector.tensor_reduce(
                        partials[:, col0 : col0 + len(items)],
                        ps[:, 0:w].rearrange("p (c n) -> p c n", n=CHUNK),
                        axis=mybir.AxisListType.X,
                        op=mybir.AluOpType.add,
                        apply_absolute_value=True,
                    )

            out_sb = const_pool.tile([128, 2], f32)
            nc.vector.reduce_sum(
                out_sb[:, 0:1], partials[:], axis=mybir.AxisListType.X
            )
            nc.vector.reduce_sum(
                out_sb[:, 1:2], nzt[:], axis=mybir.AxisListType.X
            )
            nc.sync.dma_start(out_dram.ap(), out_sb[:])

    nc.compile()
    _PROGRAM_CACHE[key] = nc
    return nc


def _preprocess(gt_points, cos_similarity, threshold):
    """Host O(B*N^2) prep: u vectors, compaction, padding.

    Returns (in_maps, nnz, P, COLS)."""
    gt = np.asarray(gt_points, dtype=np.float32)
    cos = np.asarray(cos_similarity, dtype=np.float32)
    th = np.asarray(threshold, dtype=np.float32).reshape(-1)[0]
    B, N, D = gt.shape
    M = N * N

    sm = np.where(cos < th, np.float32(0), cos)
    v = ((gt[:, :, None, :] - gt[:, None, :, :]) * sm[..., None]).reshape(B, M, D)
    v = v.astype(np.float32)
    # per-element eps, summed like the reference: (vx^2+eps) + (vy^2+eps)
    r2 = (v[..., 0] * v[..., 0] + EPS) + (v[..., 1] * v[..., 1] + EPS)
    vabs = np.sqrt(r2, dtype=np.float32)
    u = (v / vabs[..., None]).astype(np.float32)
    nz = np.any(v != 0, axis=-1)  # [B, M]
    nnz = nz.sum(axis=1).astype(np.int64)

    P = int(-(-max(1, nnz.max()) // 128) * 128)  # round up to 128, min 128
    COLS = int(-(-P // CHUNK) * CHUNK)           # round up to CHUNK
    n_tiles = P // 128

    in_maps = []
    for b in range(B):
        ub = u[b][nz[b]]  # [nnz_b, 2] compacted
        ucols = np.zeros((2, COLS), dtype=np.float32)
        ucols[:, : ub.shape[0]] = ub.T
        nzv = np.zeros(n_tiles * 128, dtype=np.float32)
        nzv[: ub.shape[0]] = 1.0
        nzdev = np.ascontiguousarray(nzv.reshape(n_tiles, 128).T)
        in_maps.append({"u": ucols, "nz": nzdev})
    return in_maps, nnz, P, COLS


def _ensure_ntff_hook():
    """Shim antenv.axon_hooks if the image lacks it (profiling only).

    Replicates trn_agent_boot._ntff_profile_via_ctypes against the baked
    libaxon_pjrt.so so run_bass_kernel_spmd(trace=True) can capture NTFFs.
    """
    try:
        from antenv.axon_hooks import get_axon_ntff_profile_hook  # noqa: F401

        return
    except ImportError:
        pass

    import contextlib
    import ctypes
    import sys
    import types

    import antenv

    mod = types.ModuleType("antenv.axon_hooks")
    _state = {"hook": None}

    def set_axon_ntff_profile_hook(h):
        _state["hook"] = h

    def get_axon_ntff_profile_hook():
        return _state["hook"]

    mod.set_axon_ntff_profile_hook = set_axon_ntff_profile_hook
    mod.get_axon_ntff_profile_hook = get_axon_ntff_profile_hook
    sys.modules["antenv.axon_hooks"] = mod
    antenv.axon_hooks = mod

    so_path = "/opt/axon/libaxon_pjrt.so"
    if not os.path.exists(so_path):
        return
    lib = ctypes.CDLL(so_path)
    if not hasattr(lib, "axon_start_nrt_profile"):
        return
    lib.axon_start_nrt_profile.argtypes = [
        ctypes.POINTER(ctypes.c_int64),
        ctypes.c_size_t,
    ]
    lib.axon_start_nrt_profile.restype = ctypes.c_int64
    lib.axon_stop_nrt_profile.argtypes = [ctypes.c_char_p]
    lib.axon_stop_nrt_profile.restype = ctypes.c_int64

    @contextlib.contextmanager
    def _hook(output_dir, device_ids):
        import jax

        jax.devices()
        if device_ids:
            ids = (ctypes.c_int64 * len(device_ids))(*device_ids)
            rc = lib.axon_start_nrt_profile(ids, len(device_ids))
        else:
            rc = lib.axon_start_nrt_profile(None, 0)
        if rc != 0:
            raise RuntimeError(f"axon_start_nrt_profile rc={rc}")
        try:
            yield
        finally:
            n = lib.axon_stop_nrt_profile(str(output_dir).encode())
            if n < 0:
                raise RuntimeError(f"axon_stop_nrt_profile rc={n}")
            print(f"profile: {n} file(s) written to {output_dir}")

    set_axon_ntff_profile_hook(_hook)


def kernel(gt_points, cos_similarity, threshold):
    global LAST_RESULTS
    in_maps, nnz, P, COLS = _preprocess(gt_points, cos_similarity, threshold)
    B = len(in_maps)

    total_count = int((nnz.astype(np.int64) ** 2).sum())
    if total_count == 0:
        # dot is identically zero: reference computes 0/0 in fp32.
        with np.errstate(invalid="ignore", divide="ignore"):
            return (np.float32(0) / np.float32(0)).astype(np.float32)

    from concourse.bass_utils import run_bass_kernel_spmd

    nc = _build_program(P, COLS)
    assert B <= N_CORES, "one batch per core"
    trace = os.environ.get("KERNEL_TRACE", "") not in ("", "0")
    if trace:
        _ensure_ntff_hook()
    res = run_bass_kernel_spmd(
        nc,
        in_maps,
        core_ids=list(range(B)),
        trace=trace,
    )
    LAST_RESULTS = res

    total = 0.0
    dev_count = 0
    for b in range(B):
        out = res.results[b]["out"]
        total += float(np.sum(out[:, 0], dtype=np.float64))
        nzs = int(round(float(np.sum(out[:, 1], dtype=np.float64))))
        dev_count += nzs * nzs
    assert dev_count == total_count, (dev_count, total_count)

    return np.asarray(
        np.float32(total) / np.float32(total_count), dtype=np.float32
    )


# revision 10
# speedup vs baseline: 2.4232x; 2.4232x over previous
"""Trainium2 Bass kernel for ComputeVecSimilarityLoss.

Reference semantics (B batches, N points, D=2):
    sm      = where(cos < th, 0, cos)                      [B,N,N]
    v[i,j]  = (gt[i] - gt[j]) * sm[i,j]  -> [B, M=N*N, D]
    dot     = v @ v^T per batch                            [B,M,M]
    idx_num = count(dot != 0)
    vabs    = sqrt(sum(v*v + 1e-9, axis=D))
    result  = sum(|dot| / (vabs_m*vabs_n)) / idx_num

Restructuring used here (mathematically exact, fp-equal to ~1e-6):
  * u = v / vabs  (host, O(B*M)): |dot|/(vabs_m*vabs_n) == |u_m . u_n|.
  * zero vectors (v == 0) contribute exactly 0.0 to every sum term and
    have dot == 0 for every pair, so they are compacted away on the host.
    idx_num = sum_b (nnz_b)^2  (the residual pairs have |dot| >> 0, no
    exact-cancellation; verified against the reference count).
  * batch b -> NeuronCore b (pure data parallel, B == 8 cores).
    Each core computes S_b = sum |u_m . u_n| over its compacted M_b x M_b
    block plus its nonzero count; host does the final scalar division.

Device kernel per core: PE emits K=2 matmuls (u^T u outer products) into
PSUM 256-column chunks; ScalarE (Abs activation with accum_out) and
VectorE (tensor_reduce with apply_absolute_value) split the |.|+sum work
over PSUM supertiles; per-partition partials are reduced on-chip to a
[128, 2] output (abs-dot partial sums, nz partial sums).
"""

import os

import numpy as np

EPS = np.float32(1e-9)
CHUNK = 256          # matmul free-dim per PSUM chunk (half bank of fp32)
GROUP_CHUNKS = 4     # chunks per PSUM supertile (2 banks = 1024 fp32)
N_CORES = 8
N_ROWGROUPS = int(os.environ.get("KERNEL_ROWGROUPS", "4"))

# Stash of the most recent BassKernelResults (for test harness profiling).
LAST_RESULTS = None

_PROGRAM_CACHE = {}


def _plan_groups(n_tiles, n_chunks):
    """Plan triangular work and consumer assignment.

    The M x M |dot| matrix is symmetric.  Row tiles are processed in
    pairs p = {2p, 2p+1}; chunk index c (CHUNK=256 columns = 2 row
    tiles wide):
      * c == p: the 2x2 "superdiagonal" block, computed once, weight 1
        (handled on ScalarE with scale=0.5 so everything sums uniformly
        and the host multiplies the grand total by 2).
      * c > p: strictly above the superdiagonal, weight 2 (scale 1.0).
    Blocks below the superdiagonal are never computed.

    Groups of up to GROUP_CHUNKS chunks share one PSUM supertile and one
    consumer instruction.  Returns (plan, npart) where plan entries are
    (items, engine, col0, scale).
    """
    n_pairs = n_tiles // 2
    d_work = [(2 * p + d, p) for p in range(n_pairs) for d in (0, 1)]
    u_work = [
        (2 * p + d, c)
        for p in range(n_pairs)
        for c in range(p + 1, n_chunks)
        for d in (0, 1)
    ]

    def cost_act(n):
        return (172.0 + CHUNK * n) / 1.2 + 283.0

    def cost_dve(n):
        return (120.0 + CHUNK * n) / 0.96

    plan = []
    col = 0
    act_ns = 0.0
    dve_ns = 0.0
    # superdiagonal groups: ScalarE with scale 0.5
    for i in range(0, len(d_work), GROUP_CHUNKS):
        items = d_work[i : i + GROUP_CHUNKS]
        plan.append((items, "act", col, 0.5))
        act_ns += cost_act(len(items))
        col += 1
    # upper groups: balance between ScalarE and VectorE
    for i in range(0, len(u_work), GROUP_CHUNKS):
        items = u_work[i : i + GROUP_CHUNKS]
        if act_ns + cost_act(len(items)) <= dve_ns + cost_dve(len(items)):
            plan.append((items, "act", col, 1.0))
            act_ns += cost_act(len(items))
            col += 1
        else:
            plan.append((items, "dve", col, 1.0))
            dve_ns += cost_dve(len(items))
            col += len(items)
    return plan, col


def _build_program(P, COLS):
    """Build (and cache) the Bass program for padded size P x COLS."""
    key = (P, COLS)
    if key in _PROGRAM_CACHE:
        return _PROGRAM_CACHE[key]

    import concourse.bass as bass
    import concourse.mybir as mybir
    import concourse.tile as tile
    from concourse import bacc
    from concourse.bass import ts

    f32 = mybir.dt.float32
    f16 = mybir.dt.float16
    n_tiles = P // 128
    n_chunks = COLS // CHUNK
    plan, npart = _plan_groups(n_tiles, n_chunks)

    nc = bacc.Bacc(
        "TRN2",
        target_bir_lowering=False,
        debug=False,
        enable_asserts=False,
        num_devices=N_CORES,
    )
    u_dram = nc.dram_tensor("u", [2, COLS], f16, kind="ExternalInput")
    nz_dram = nc.dram_tensor("nz", [128, n_tiles], f32, kind="ExternalInput")
    out_dram = nc.dram_tensor("out", [128, 2], f32, kind="ExternalOutput")

    with tile.TileContext(nc) as tc:
        with (
            tc.tile_pool(name="const", bufs=1) as const_pool,
            tc.tile_pool(name="psum", bufs=4, space="PSUM") as psum_pool,
        ):
            # u replicated at partition bases 0/32/64/96 so consecutive
            # matmuls land in distinct PE row groups and run concurrently.
            u = const_pool.tile([128, COLS], f16)
            for r in range(4):
                nc.sync.dma_start(u[32 * r : 32 * r + 2, :], u_dram.ap())
            nzt = const_pool.tile([128, n_tiles], f32)
            nc.gpsimd.dma_start(nzt[:], nz_dram.ap())
            partials = const_pool.tile([128, npart], f32)

            mm_idx = 0
            for items, engine, col0, scale in plan:
                w = CHUNK * len(items)
                ps = psum_pool.tile([128, GROUP_CHUNKS * CHUNK], f32, tag="ps")
                for j, (t, c) in enumerate(items):
                    r = 32 * (mm_idx % N_ROWGROUPS)
                    mm_idx += 1
                    nc.tensor.matmul(
                        ps[:, ts(j, CHUNK)],
                        u[r : r + 2, ts(t, 128)],
                        u[r : r + 2, ts(c, CHUNK)],
                        tile_position=(r, 0),
                    )
                if engine == "act":
                    nc.scalar.activation(
                        ps[:, 0:w],
                        ps[:, 0:w],
                        mybir.ActivationFunctionType.Abs,
                        scale=scale,
                        accum_out=partials[:, col0 : col0 + 1],
                    )
                else:
                    assert scale == 1.0
                    nc.vector.tensor_reduce(
                        partials[:, col0 : col0 + len(items)],
                        ps[:, 0:w].rearrange("p (c n) -> p c n", n=CHUNK),
                        axis=mybir.AxisListType.X,
                        op=mybir.AluOpType.add,
                        apply_absolute_value=True,
                    )

            out_sb = const_pool.tile([128, 2], f32)
            nc.vector.reduce_sum(
                out_sb[:, 0:1], partials[:], axis=mybir.AxisListType.X
            )
            nc.vector.reduce_sum(
                out_sb[:, 1:2], nzt[:], axis=mybir.AxisListType.X
            )
            nc.sync.dma_start(out_dram.ap(), out_sb[:])

    nc.compile()
    _PROGRAM_CACHE[key] = nc
    return nc


def _preprocess(gt_points, cos_similarity, threshold):
    """Host O(B*N^2) prep: u vectors, compaction, padding.

    Returns (in_maps, nnz, P, COLS)."""
    gt = np.asarray(gt_points, dtype=np.float32)
    cos = np.asarray(cos_similarity, dtype=np.float32)
    th = np.asarray(threshold, dtype=np.float32).reshape(-1)[0]
    B, N, D = gt.shape
    M = N * N

    sm = np.where(cos < th, np.float32(0), cos)
    v = ((gt[:, :, None, :] - gt[:, None, :, :]) * sm[..., None]).reshape(B, M, D)
    v = v.astype(np.float32)
    # per-element eps, summed like the reference: (vx^2+eps) + (vy^2+eps)
    r2 = (v[..., 0] * v[..., 0] + EPS) + (v[..., 1] * v[..., 1] + EPS)
    vabs = np.sqrt(r2, dtype=np.float32)
    u = (v / vabs[..., None]).astype(np.float32)
    nz = np.any(v != 0, axis=-1)  # [B, M]
    nnz = nz.sum(axis=1).astype(np.int64)

    # round up to CHUNK (=256) so the row-tile pair scheme always has an
    # even tile count; COLS == P.
    P = int(-(-max(1, nnz.max()) // CHUNK) * CHUNK)
    COLS = P
    n_tiles = P // 128

    in_maps = []
    for b in range(B):
        ub = u[b][nz[b]]  # [nnz_b, 2] compacted
        ucols = np.zeros((2, COLS), dtype=np.float16)
        ucols[:, : ub.shape[0]] = ub.T.astype(np.float16)
        nzv = np.zeros(n_tiles * 128, dtype=np.float32)
        nzv[: ub.shape[0]] = 1.0
        nzdev = np.ascontiguousarray(nzv.reshape(n_tiles, 128).T)
        in_maps.append({"u": ucols, "nz": nzdev})
    return in_maps, nnz, P, COLS


def _ensure_ntff_hook():
    """Shim antenv.axon_hooks if the image lacks it (profiling only).

    Replicates trn_agent_boot._ntff_profile_via_ctypes against the baked
    libaxon_pjrt.so so run_bass_kernel_spmd(trace=True) can capture NTFFs.
    """
    try:
        from antenv.axon_hooks import get_axon_ntff_profile_hook  # noqa: F401

        return
    except ImportError:
        pass

    import contextlib
    import ctypes
    import sys
    import types

    import antenv

    mod = types.ModuleType("antenv.axon_hooks")
    _state = {"hook": None}

    def set_axon_ntff_profile_hook(h):
        _state["hook"] = h

    def get_axon_ntff_profile_hook():
        return _state["hook"]

    mod.set_axon_ntff_profile_hook = set_axon_ntff_profile_hook
    mod.get_axon_ntff_profile_hook = get_axon_ntff_profile_hook
    sys.modules["antenv.axon_hooks"] = mod
    antenv.axon_hooks = mod

    so_path = "/opt/axon/libaxon_pjrt.so"
    if not os.path.exists(so_path):
        return
    lib = ctypes.CDLL(so_path)
    if not hasattr(lib, "axon_start_nrt_profile"):
        return
    lib.axon_start_nrt_profile.argtypes = [
        ctypes.POINTER(ctypes.c_int64),
        ctypes.c_size_t,
    ]
    lib.axon_start_nrt_profile.restype = ctypes.c_int64
    lib.axon_stop_nrt_profile.argtypes = [ctypes.c_char_p]
    lib.axon_stop_nrt_profile.restype = ctypes.c_int64

    @contextlib.contextmanager
    def _hook(output_dir, device_ids):
        import jax

        jax.devices()
        if device_ids:
            ids = (ctypes.c_int64 * len(device_ids))(*device_ids)
            rc = lib.axon_start_nrt_profile(ids, len(device_ids))
        else:
            rc = lib.axon_start_nrt_profile(None, 0)
        if rc != 0:
            raise RuntimeError(f"axon_start_nrt_profile rc={rc}")
        try:
            yield
        finally:
            n = lib.axon_stop_nrt_profile(str(output_dir).encode())
            if n < 0:
                raise RuntimeError(f"axon_stop_nrt_profile rc={n}")
            print(f"profile: {n} file(s) written to {output_dir}")

    set_axon_ntff_profile_hook(_hook)


def kernel(gt_points, cos_similarity, threshold):
    global LAST_RESULTS
    in_maps, nnz, P, COLS = _preprocess(gt_points, cos_similarity, threshold)
    B = len(in_maps)

    total_count = int((nnz.astype(np.int64) ** 2).sum())
    if total_count == 0:
        # dot is identically zero: reference computes 0/0 in fp32.
        with np.errstate(invalid="ignore", divide="ignore"):
            return (np.float32(0) / np.float32(0)).astype(np.float32)

    from concourse.bass_utils import run_bass_kernel_spmd

    nc = _build_program(P, COLS)
    assert B <= N_CORES, "one batch per core"
    trace = os.environ.get("KERNEL_TRACE", "") not in ("", "0")
    if trace:
        _ensure_ntff_hook()
    res = run_bass_kernel_spmd(
        nc,
        in_maps,
        core_ids=list(range(B)),
        trace=trace,
    )
    LAST_RESULTS = res

    total = 0.0
    dev_count = 0
    for b in range(B):
        out = res.results[b]["out"]
        # partials hold (upper + 0.5*superdiag); x2 recovers the full sum
        total += 2.0 * float(np.sum(out[:, 0], dtype=np.float64))
        nzs = int(round(float(np.sum(out[:, 1], dtype=np.float64))))
        dev_count += nzs * nzs
    assert dev_count == total_count, (dev_count, total_count)

    return np.asarray(
        np.float32(total) / np.float32(total_count), dtype=np.float32
    )
